# revision 5
# baseline (speedup 1.0000x reference)
"""GRU sequence model kernel for Trainium2 (8 NeuronCores, data-parallel).

Computes, per core (batch shard of 512):
    gi = x @ w_ih.T + b_ih            # done per-timestep, fused in loop
    h_{t+1} = GRU-cell(gi_t, h_t)     # 50 steps, hidden 512
    out = h_T @ w_out.T + b_out

Layout strategy: hidden state and all gate tensors live transposed on chip
([gate/hidden dim on partitions, batch on free dim]) so the recurrent matmul,
activations and elementwise updates need no per-step transposes. Only x_t is
transposed (PE transpose-mode). All matmuls run as float32r (full PE rate).
"""

import sys
from contextlib import ExitStack

import numpy as np

sys.path.insert(0, "/opt/trn_rl_repo")

import concourse.bass as bass  # noqa: E402
import concourse.tile as tile  # noqa: E402
from concourse import bacc, mybir  # noqa: E402
from concourse.bass_utils import run_bass_kernel_spmd  # noqa: E402

P = 128
T_STEPS = 50
B_LOCAL = 512  # batch per core
I_DIM = 256  # input dim  (2 k-chunks)
H_DIM = 512  # hidden dim (4 k-chunks)
G_DIM = 1536  # 3*H gates  (12 chunks)
O_DIM = 256  # output dim
N_CORES = 8

F32 = mybir.dt.float32
F32R = mybir.dt.float32r
AF = mybir.ActivationFunctionType
ALU = mybir.AluOpType


def _r(ap):
    """Matmul operand tiles are declared float32r; passthrough."""
    return ap


def _emit(ctx: ExitStack, tc: tile.TileContext, x_d, wih_d, whh_d, wout_d, bias_d, ident_d, ident_f32_d, out_d, n_steps):
    nc = tc.nc
    KI = I_DIM // P  # 2
    KH = H_DIM // P  # 4
    NB = B_LOCAL // P  # 4 batch chunks

    consts = ctx.enter_context(tc.tile_pool(name="consts", bufs=1))
    xload = ctx.enter_context(tc.tile_pool(name="xload", bufs=3))
    xtp = ctx.enter_context(tc.tile_pool(name="xtp", bufs=2))
    gates = ctx.enter_context(tc.tile_pool(name="gates", bufs=3))
    ps_rz = ctx.enter_context(tc.tile_pool(name="ps_rz", bufs=3, space="PSUM"))
    ps_in = ctx.enter_context(tc.tile_pool(name="ps_in", bufs=2, space="PSUM"))
    ps_hn = ctx.enter_context(tc.tile_pool(name="ps_hn", bufs=1, space="PSUM"))
    ps_xt = ctx.enter_context(tc.tile_pool(name="ps_xt", bufs=2, space="PSUM"))

    # --- persistent SBUF tensors ---
    w_ih = consts.tile([P, KI, G_DIM], F32R, tag="w_ih")
    nc.sync.dma_start(w_ih[:], wih_d.rearrange("(ko p) g -> p ko g", p=P))
    w_hh = consts.tile([P, KH, G_DIM], F32R, tag="w_hh")
    nc.sync.dma_start(w_hh[:], whh_d.rearrange("(ko p) g -> p ko g", p=P))
    w_out = consts.tile([P, KH, O_DIM], F32R, tag="w_out")
    nc.sync.dma_start(w_out[:], wout_d.rearrange("(ko p) g -> p ko g", p=P))
    biases = consts.tile([P, 18], F32, tag="biases")
    nc.sync.dma_start(biases[:], bias_d)
    ident = consts.tile([P, P], F32R, tag="ident")
    nc.sync.dma_start(ident[:], ident_d)
    ident_f32 = consts.tile([P, P], F32, tag="ident_f32")
    nc.sync.dma_start(ident_f32[:], ident_f32_d)

    # double-buffered hidden state, transposed layout [h-dim, batch]
    hbuf = [
        consts.tile([P, KH, B_LOCAL], F32R, tag=f"hbuf{i}", name=f"hbuf{i}")
        for i in range(2)
    ]

    for t in range(n_steps):
        h_rd = hbuf[t % 2]
        h_wr = hbuf[(t + 1) % 2]

        # ---- load x_t and transpose to [i-dim, batch] ----
        x_raw = xload.tile([P, NB, I_DIM], F32R, tag="x_raw")
        for bc in range(NB):
            nc.sync.dma_start(x_raw[:, bc, :], x_d[bc * P:(bc + 1) * P, t, :])
        xT = xtp.tile([P, KI, B_LOCAL], F32R, tag="xT")
        for ic in range(KI):
            pxt = ps_xt.tile([P, B_LOCAL], F32R, tag="pxt")
            for bc in range(NB):
                nc.tensor.transpose(
                    pxt[:, bc * P:(bc + 1) * P],
                    x_raw[:, bc, ic * P:(ic + 1) * P],
                    ident,
                )
            nc.vector.tensor_copy(xT[:, ic, :], pxt[:])

        # ---- gate computation per hidden chunk ----
        for hc in range(KH):
            rc, zc, nch = hc, KH + hc, 2 * KH + hc  # gate chunk ids (of 12)

            p_hn = None
            if t > 0:
                p_hn = ps_hn.tile([P, B_LOCAL], F32, tag="p_hn")
                for kc in range(KH):
                    nc.tensor.matmul(
                        p_hn[:], _r(w_hh[:, kc, nch * P:(nch + 1) * P]), _r(h_rd[:, kc, :]),
                        start=(kc == 0), stop=(kc == KH - 1),
                    )
            p_r = ps_rz.tile([P, B_LOCAL], F32, tag="rz")
            p_z = ps_rz.tile([P, B_LOCAL], F32, tag="rz")
            for gc, pt in ((rc, p_r), (zc, p_z)):
                for ic in range(KI):
                    nc.tensor.matmul(
                        pt[:], _r(w_ih[:, ic, gc * P:(gc + 1) * P]), _r(xT[:, ic, :]),
                        start=(ic == 0), stop=(t == 0 and ic == KI - 1),
                    )
                if t > 0:
                    for kc in range(KH):
                        nc.tensor.matmul(
                            pt[:], _r(w_hh[:, kc, gc * P:(gc + 1) * P]), _r(h_rd[:, kc, :]),
                            start=False, stop=(kc == KH - 1),
                        )
            p_in = ps_in.tile([P, B_LOCAL], F32, tag="p_in")
            for ic in range(KI):
                nc.tensor.matmul(
                    p_in[:], _r(w_ih[:, ic, nch * P:(nch + 1) * P]), _r(xT[:, ic, :]),
                    start=(ic == 0), stop=(ic == KI - 1),
                )

            # r = sigmoid(p_r + b_rz[rc]) ; z likewise
            r_t = gates.tile([P, B_LOCAL], F32, tag="r")
            nc.scalar.activation(r_t[:], p_r[:], AF.Sigmoid, bias=biases[:, rc:rc + 1])
            z_t = gates.tile([P, B_LOCAL], F32, tag="z")
            nc.scalar.activation(z_t[:], p_z[:], AF.Sigmoid, bias=biases[:, zc:zc + 1])
            # rh = (p_hn + b_hh_n) * r    (at t=0, h==0 so p_hn == 0)
            rh = gates.tile([P, B_LOCAL], F32, tag="rh")
            if t > 0:
                nc.vector.scalar_tensor_tensor(
                    rh[:], p_hn[:], biases[:, 12 + hc:13 + hc], r_t[:], ALU.add, ALU.mult,
                )
            else:
                nc.vector.tensor_scalar_mul(rh[:], r_t[:], biases[:, 12 + hc:13 + hc])
            # n = tanh(rh + p_in + b_ih_n)
            pre = gates.tile([P, B_LOCAL], F32, tag="pre")
            nc.vector.tensor_add(pre[:], rh[:], p_in[:])
            n_t = gates.tile([P, B_LOCAL], F32, tag="n")
            nc.scalar.activation(n_t[:], pre[:], AF.Tanh, bias=biases[:, 8 + hc:9 + hc])
            # h_new = n + z * (h - n)    (at t=0, h==0 so d = -n)
            d_t = gates.tile([P, B_LOCAL], F32, tag="d")
            if t > 0:
                nc.gpsimd.tensor_sub(d_t[:], h_rd[:, hc, :], n_t[:])
            else:
                nc.gpsimd.tensor_scalar_mul(d_t[:], n_t[:], -1.0)
            e_t = gates.tile([P, B_LOCAL], F32, tag="e")
            nc.gpsimd.tensor_mul(e_t[:], z_t[:], d_t[:])
            nc.vector.tensor_add(h_wr[:, hc, :], n_t[:], e_t[:])

    # ---- output projection: out[b, o] = h.T @ w_out.T + b_out ----
    h_fin = hbuf[n_steps % 2]
    o_sb = []
    for oc in range(O_DIM // P):
        p_o = ps_rz.tile([P, B_LOCAL], F32, tag="rz")
        for kc in range(KH):
            nc.tensor.matmul(
                p_o[:], _r(w_out[:, kc, oc * P:(oc + 1) * P]), _r(h_fin[:, kc, :]),
                start=(kc == 0), stop=(kc == KH - 1),
            )
        ot = gates.tile([P, B_LOCAL], F32, tag=f"osb{oc}")
        nc.scalar.activation(ot[:], p_o[:], AF.Identity, bias=biases[:, 16 + oc:17 + oc])
        o_sb.append(ot)
    # transpose back to [batch, o] and store
    for bc in range(NB):
        outT = gates.tile([P, O_DIM], F32, tag="outT")
        for oc in range(O_DIM // P):
            pxt = ps_hn.tile([P, B_LOCAL], F32, tag="p_hn")
            nc.tensor.transpose(
                pxt[:, :P], o_sb[oc][:, bc * P:(bc + 1) * P], ident_f32,
            )
            nc.vector.tensor_copy(outT[:, oc * P:(oc + 1) * P], pxt[:, :P])
        nc.sync.dma_start(out_d[bc * P:(bc + 1) * P, :], outT[:])


def build_program(n_steps=T_STEPS):
    nc = bacc.Bacc("TRN2", target_bir_lowering=False, debug=False, num_devices=N_CORES)
    x_d = nc.dram_tensor("x", [B_LOCAL, n_steps, I_DIM], F32R, kind="ExternalInput").ap()
    wih_d = nc.dram_tensor("w_ih_t", [I_DIM, G_DIM], F32R, kind="ExternalInput").ap()
    whh_d = nc.dram_tensor("w_hh_t", [H_DIM, G_DIM], F32R, kind="ExternalInput").ap()
    wout_d = nc.dram_tensor("w_out_t", [H_DIM, O_DIM], F32R, kind="ExternalInput").ap()
    bias_d = nc.dram_tensor("biases", [P, 18], F32, kind="ExternalInput").ap()
    ident_d = nc.dram_tensor("ident", [P, P], F32R, kind="ExternalInput").ap()
    ident_f32_d = nc.dram_tensor("ident_f32", [P, P], F32, kind="ExternalInput").ap()
    out_d = nc.dram_tensor("out", [B_LOCAL, O_DIM], F32, kind="ExternalOutput").ap()

    with tile.TileContext(nc) as tc:
        with ExitStack() as ctx:
            _emit(ctx, tc, x_d, wih_d, whh_d, wout_d, bias_d, ident_d, ident_f32_d, out_d, n_steps)
    nc.compile()
    return nc


def make_host_inputs(x, w_ih, w_hh, b_ih, b_hh, w_out, b_out):
    """Host-side prep: transpose weights, pack biases into [128, 18]."""
    w_ih_t = np.ascontiguousarray(np.asarray(w_ih, dtype=np.float32).T)
    w_hh_t = np.ascontiguousarray(np.asarray(w_hh, dtype=np.float32).T)
    w_out_t = np.ascontiguousarray(np.asarray(w_out, dtype=np.float32).T)
    b_ih = np.asarray(b_ih, dtype=np.float32)
    b_hh = np.asarray(b_hh, dtype=np.float32)
    b_out = np.asarray(b_out, dtype=np.float32)

    bias_pack = np.zeros((P, 18), dtype=np.float32)
    b_comb = b_ih + b_hh
    for j in range(8):
        bias_pack[:, j] = b_comb[j * P:(j + 1) * P]
    for j in range(4):
        bias_pack[:, 8 + j] = b_ih[2 * H_DIM + j * P:2 * H_DIM + (j + 1) * P]
        bias_pack[:, 12 + j] = b_hh[2 * H_DIM + j * P:2 * H_DIM + (j + 1) * P]
    bias_pack[:, 16] = b_out[:P]
    bias_pack[:, 17] = b_out[P:]
    return w_ih_t, w_hh_t, w_out_t, bias_pack


_IDENT = np.eye(128, dtype=np.float32)
_CACHED_NC = None


def _get_nc():
    global _CACHED_NC
    if _CACHED_NC is None:
        _CACHED_NC = build_program()
    return _CACHED_NC


LAST_RESULT = None


def kernel(x, w_ih, w_hh, b_ih, b_hh, w_out, b_out, trace=False):
    x = np.asarray(x, dtype=np.float32)
    w_ih_t, w_hh_t, w_out_t, bias_pack = make_host_inputs(
        x, w_ih, w_hh, b_ih, b_hh, w_out, b_out
    )
    nc = _get_nc()
    in_maps = []
    for c in range(N_CORES):
        in_maps.append({
            "x": np.ascontiguousarray(x[c * B_LOCAL:(c + 1) * B_LOCAL]),
            "w_ih_t": w_ih_t,
            "w_hh_t": w_hh_t,
            "w_out_t": w_out_t,
            "biases": bias_pack,
            "ident": _IDENT,
            "ident_f32": _IDENT,
        })
    global LAST_RESULT
    LAST_RESULT = run_bass_kernel_spmd(
        nc, in_maps, core_ids=list(range(N_CORES)), trace=trace,
    )
    return np.concatenate(
        [LAST_RESULT.results[c]["out"] for c in range(N_CORES)], axis=0
    )
